# revision 30
# baseline (speedup 1.0000x reference)
"""LIF spiking-neuron scan kernel for Trainium2 (Bass/Tile), 8-core SPMD.

Reference semantics (per element, T=4 sequential steps):
    mem = 0
    for t in range(T):
        mem = mem + x[t]
        s[t] = (mem >= 1.0)          # spike, exact 0.0/1.0 fp32
        mem = mem * (mem < 1.0)      # hard reset on spike
All membrane math is fp32 and bit-exact vs the jax reference.

Sharding: x is [T*B, C, H, W] = [256, 128, 32, 32] fp32. Reshaped to
[T=4, B=64, C*H*W]; B is data-parallel sharded 8 ways. Each core's shard
is viewed as [T, 128, 8192] fp32. The T-scan is local per core.

v2 "packed" design (vs the 73-75us baseline that stored one int8 sgn per
(t, elem) = 4 MiB/core):
  - DVE: the irreducible 6-op fp32 chain per block
    (stt0, add1, stt1, add2, stt2, add3), ~56us busy.
  - act: sgn_t = Sign(u_t - 1) in bf16 {-1,0,1}, plus the final
    PSUM->int8 cast.
  - PE (idle in the baseline): packs all 4 timesteps into ONE int8 via
    4 accumulating matmuls with scaled-identity weights:
        packed = sum_t 4^t * sgn_t  in [-85, 85]
    Balanced base-4: digits in {-1,0,1} with radix 4 are uniquely
    decodable; every product 4^t*sgn_t and the <=4-term fp32 PSUM sum
    is exact. Host decodes with a 256-entry LUT (spike_t = digit_t >= 0,
    which also absorbs hw Sign(+0) returning 0 or 1).
  - Output DMA drops 4 MiB -> 1 MiB/core; DMA active ~62us -> ~53us.
Block-major emission (t innermost) keeps PSUM pack-bank lifetimes to one
block so 2 in-flight blocks fit the 8 banks.
"""

import numpy as np

import concourse.bacc as bacc
import concourse.mybir as mybir
import concourse.tile as tile
from concourse.bass_utils import run_bass_kernel_spmd

T = 4
B = 64
CHW = 128 * 32 * 32  # 131072
N_CORES = 8
B_SHARD = B // N_CORES           # 8
ELEMS = B_SHARD * CHW            # 1048576 elems per timestep per core
P = 128
F = ELEMS // P                   # 8192
MM_W = 512                       # one PSUM bank of fp32 / max moving dim

_cache = {}


PLAN = ((128, "raw"), (1024, "pack"), (2048, "pack"), (2048, "pack"),
        (1536, "pack"), (896, "raw"), (512, "raw"))
PACK_CHUNK = 2048                # psum tile width (4 banks)
RAW_TOT = sum(w for w, m in PLAN if m == "raw")
PACK_TOT = sum(w for w, m in PLAN if m == "pack")


def _build_pack_module(plan=PLAN, sgn_bufs=3, out_bufs=4, mem_bufs=2,
                       psum_bufs=2, cast_lag=4, stt0_pool=False,
                       tail_sync_stores=2):
    """Plan-based builder. Each plan entry (width, mode) is one load
    block (a single big DMA per t -> near-peak HWDGE streaming), one
    DVE/ACT compute block (full width ops), and for mode=="pack" a set
    of <=PACK_CHUNK psum chunks.

    mode=="pack": spikes leave as ONE int8 per element, packed by the PE
    as balanced base-4 digits: ACT writes sgn_t = Sign(u_t-1) as fp8e4
    pairs in [128, 2, w] tiles; two DoubleRow fp8 matmuls per 512-column
    chunk (contraction 2x128, 0.5 cyc/row) accumulate
        psum  = [I; 4I]^T   @ [sgn0; sgn1]
        psum += [16I; 64I]^T @ [sgn2; sgn3]
    exactly ({-1,0,1}x{1,4,16,64}, |sum|<=85), then ACT casts psum to
    int8 and the result is stored (1 byte per 4 spikes).

    mode=="raw": signs are stored unpacked (int8 sgn per t, 4 bytes per
    4 spikes). Used for the ramp block (smallest possible first
    dependency) and the tail blocks, where the pack pipeline
    (sign->matmul->cast->store) would sit on the critical path after the
    last DVE op.

    cast_lag: number of subsequent sign emissions before a pending
    psum->int8 cast is released to the (in-order) ACT queue, so casts
    never stall ACT waiting on the pack matmuls."""
    assert sum(w for w, _ in plan) == F
    for w, m in plan:
        assert m != "pack" or w % MM_W == 0

    fp32 = mybir.dt.float32
    fp8 = mybir.dt.float8e4
    int8 = mybir.dt.int8
    Alu = mybir.AluOpType

    nc = bacc.Bacc("TRN2", target_bir_lowering=False, debug=False)
    x = nc.dram_tensor("x", (T, P, F), fp32, kind="ExternalInput").ap()
    # w[:, t, :] = 4^t * I_128 (fp8e4; exact small powers)
    w = nc.dram_tensor("w", (P, T, P), fp8, kind="ExternalInput").ap()
    out = nc.dram_tensor("out", (P, PACK_TOT), int8,
                         kind="ExternalOutput").ap()
    out2 = nc.dram_tensor("out2", (T, P, RAW_TOT), int8,
                          kind="ExternalOutput").ap()

    with tile.TileContext(nc) as tc:
        with (
            tc.tile_pool(name="xp", bufs=1) as xpool,
            tc.tile_pool(name="mp", bufs=mem_bufs) as mpool,
            tc.tile_pool(name="gp", bufs=sgn_bufs) as gpool,
            tc.tile_pool(name="op", bufs=out_bufs) as opool,
            tc.tile_pool(name="cp", bufs=1) as cpool,
            tc.tile_pool(name="pp", bufs=psum_bufs, space="PSUM") as ppool,
        ):
            w_sb = cpool.tile([P, T, P], fp8, tag="w", bufs=1)
            neg1 = cpool.tile([P, 1], fp32, tag="neg1", bufs=1)
            nc.vector.memset(neg1[:], -1.0)

            # pending: (psum chunk, out tile, out col slice) awaiting cast
            pending = []
            n_signs_since_pend = 0

            def flush_pending(force=False):
                nonlocal n_signs_since_pend
                while pending and (force or n_signs_since_pend >= cast_lag):
                    psum_t, out_t, osl = pending.pop(0)
                    # PSUM fp32 in [-85, 85] -> int8 SBUF (exact)
                    nc.scalar.copy(out_t[:], psum_t[:])
                    nc.scalar.dma_start(out=out[:, osl], in_=out_t[:])
                    if not force:
                        break

            foff = poff = roff = 0
            for li, (wdt, mode) in enumerate(plan):
                is_last = li == len(plan) - 1
                lsl = slice(foff, foff + wdt)
                xts = []
                for t in range(T):
                    # All of x stays resident in SBUF (128KB/partition):
                    # loads never wait on compute to free a slot, and one
                    # big DMA per (t, block) keeps the single HWDGE queue
                    # near its peak rate (~355-400 GB/s; it drops to
                    # ~278 at 4KB rows).
                    xt = xpool.tile([P, wdt], fp32, tag=f"x{li}_{t}",
                                    bufs=1)
                    nc.sync.dma_start(out=xt[:], in_=x[t, :, lsl])
                    xts.append(xt)
                if li == 0:
                    # weights load deferred behind the ramp block's x
                    # loads (first matmul is ~8us away).
                    nc.sync.dma_start(out=w_sb[:], in_=w[:, :, :])
                mem = mpool.tile([P, wdt], fp32, tag="mem")
                if mode == "pack":
                    chunks = [(c, min(PACK_CHUNK, wdt - c))
                              for c in range(0, wdt, PACK_CHUNK)]
                    psums = {}
                    for c, cw in chunks:
                        pt = ppool.tile([P, cw], fp32, tag="pk")
                        psums[c] = pt
                    g01 = gpool.tile([P, 2, wdt], fp8, tag="g")
                    g23 = gpool.tile([P, 2, wdt], fp8, tag="g")
                    gpair = {0: g01, 1: g23}
                for t in range(T):
                    if t == 0:
                        u = xts[0][:]
                    else:
                        # u computed in-place over x_t (dead after)
                        u = xts[t][:]
                        nc.vector.tensor_add(u, mem[:], u)
                    if mode == "raw":
                        st = opool.tile([P, wdt], int8, tag="o2")
                        nc.scalar.sign(st[:], u, bias=neg1[:])
                        # Tail raw stores go on the sync queue (idle
                        # once load dispatch is done) so their ~0.6us
                        # dispatches never pace the ACT sign chain.
                        q = (nc.sync if li >= len(plan) - tail_sync_stores
                             else nc.scalar)
                        q.dma_start(out=out2[t, :, roff:roff + wdt],
                                    in_=st[:])
                    else:
                        sgn = gpair[t // 2][:, t % 2, :]
                        nc.scalar.sign(sgn, u, bias=neg1[:])
                    n_signs_since_pend += 1
                    flush_pending()
                    if t < T - 1:
                        # mem' = (u < 1) * u ; the t=0 reset of pack
                        # blocks can run on the (otherwise idle) Pool
                        # engine to shave the DVE critical chain.
                        eng = (nc.gpsimd if (stt0_pool and t == 0
                                             and mode == "pack")
                               else nc.vector)
                        eng.scalar_tensor_tensor(
                            mem[:], u, 1.0, u, Alu.is_lt, Alu.mult)
                    if mode == "pack" and t % 2 == 1:
                        # DoubleRow pack of the finished sgn pair
                        pair = t // 2
                        for c, cw in chunks:
                            for k in range(cw // MM_W):
                                ks = slice(k * MM_W, (k + 1) * MM_W)
                                gks = slice(c + k * MM_W,
                                            c + (k + 1) * MM_W)
                                nc.tensor.matmul(
                                    psums[c][:, ks],
                                    w_sb[:, 2 * pair:2 * pair + 2, :],
                                    gpair[pair][:, :, gks],
                                    start=(pair == 0),
                                    stop=(pair == 1),
                                    perf_mode=mybir.MatmulPerfMode.DoubleRow,
                                )
                if mode == "pack":
                    for c, cw in chunks:
                        ot = opool.tile([P, cw], int8, tag="o")
                        pending.append(
                            (psums[c], ot, slice(poff + c, poff + c + cw)))
                    n_signs_since_pend = 0
                    poff += wdt
                else:
                    roff += wdt
                foff += wdt
            flush_pending(force=True)
    nc.compile()
    return nc


def _get_module():
    if "nc" not in _cache:
        _cache["nc"] = _build_pack_module()
    return _cache["nc"]


def _pack_weights():
    # [128, 4, 128] fp8e4: w[:, t, :] = 4^t * I
    import ml_dtypes
    wt = np.zeros((P, T, P), dtype=np.float32)
    for t in range(T):
        wt[:, t, :] = (4.0 ** t) * np.eye(P, dtype=np.float32)
    return wt.astype(ml_dtypes.float8_e4m3fn)


def _decode_lut():
    # packed = sum_t 4^t d_t, d_t in {-1,0,1}; spike_t = (d_t >= 0)
    lut = np.zeros((256, T), dtype=np.float32)
    for d0 in (-1, 0, 1):
        for d1 in (-1, 0, 1):
            for d2 in (-1, 0, 1):
                for d3 in (-1, 0, 1):
                    p = d0 + 4 * d1 + 16 * d2 + 64 * d3
                    lut[p & 0xFF] = [d0 >= 0, d1 >= 0, d2 >= 0, d3 >= 0]
    return lut


def _shard_inputs(x_np):
    # x_np: [T*B, C, H, W] fp32 -> per-core [T, P, F]
    xr = np.ascontiguousarray(x_np).reshape(T, B, CHW)
    shards = []
    for k in range(N_CORES):
        sh = np.ascontiguousarray(xr[:, k * B_SHARD : (k + 1) * B_SHARD]).reshape(
            T, P, F
        )
        shards.append(sh)
    return shards


def _unshard_outputs(outs):
    # outs: list of ([P, PACK_TOT] packed int8, [T, P, RAW_TOT] sgn int8)
    # -> [T*B, C, H, W] fp32
    lut = _decode_lut()
    full = np.empty((T, B, CHW), dtype=np.float32)
    for k, (o, o2) in enumerate(outs):
        dec = np.empty((T, P, F), dtype=np.float32)
        foff = poff = roff = 0
        o = o.reshape(P, PACK_TOT)
        o2 = o2.reshape(T, P, RAW_TOT)
        for wdt, mode in PLAN:
            if mode == "pack":
                body = lut[o[:, poff:poff + wdt].view(np.uint8)]
                dec[:, :, foff:foff + wdt] = np.moveaxis(body, -1, 0)
                poff += wdt
            else:
                dec[:, :, foff:foff + wdt] = o2[:, :, roff:roff + wdt] >= 0
                roff += wdt
            foff += wdt
        full[:, k * B_SHARD : (k + 1) * B_SHARD] = dec.reshape(
            T, B_SHARD, CHW)
    return full.reshape(T * B, 128, 32, 32)


def _in_maps(x_np):
    w_np = _pack_weights()
    return [{"x": sh, "w": w_np} for sh in _shard_inputs(x_np)]


def kernel(x, T=4, **_unused):
    x_np = np.asarray(x, dtype=np.float32)
    assert int(T) == 4, f"kernel hardcoded for T=4, got {T}"
    assert x_np.shape == (256, 128, 32, 32), x_np.shape

    nc = _get_module()
    in_maps = _in_maps(x_np)
    res = run_bass_kernel_spmd(nc, in_maps, list(range(N_CORES)))
    outs = [(r["out"], r["out2"]) for r in res.results]
    return _unshard_outputs(outs)
